# revision 14
# baseline (speedup 1.0000x reference)
"""MoE top-1 routing kernel for Trainium2 (8 NeuronCores, data-parallel).

Computes, for each token t:
    clean   = input[t] @ w_gate                    # [3]
    raw     = input[t] @ w_noise                   # [3]
    logits  = clean + noise[t] * (softplus(raw) + 0.2)
    out[t]  = argmax(logits)                       # int32, first-max tie-break

Sharding: token dim split evenly across 8 cores; weights replicated.

Design (v2, DMA-roofline): the host pre-casts the input to fp16 and
pre-transposes it to [D, NPC] per core, so the device does NO input
transposes at all -- the 16 MiB/core fp16 input streams in as fully
contiguous DMA (8 KiB per partition line) at HBM rate, and everything
else hides under it.

Weights stay fp32-exact on device via an fp16 hi+lo split: the [128, 128]
stationary for (band jj, k-chunk) holds [wg_hi|wn_hi|wg_lo|wn_lo] (12
cols) at column offset 16*jj, zeros elsewhere. Accumulating the 16
matmuls (8 bands x 2 k-chunks) of a tile into one PSUM bank stacks 8
blocks' logits at partition bands 16*jj: out rows 16jj+y with
y = [c_hi(3), r_hi(3), c_lo(3), r_lo(3), pad(4)].

Per tile u (4096 tokens, 8 per core):
  - DMA in_t [128, (k t)] fp16 (2 MiB, alternating sync/scalar HWDGE)
  - 16 accumulating matmuls -> L [128, 512] PSUM   (tokens n = 128c + p)
  - ACT copy L -> SBUF, 4 PE transposes -> F [128, 512] PSUM
    (token p on partitions, free = 128c + 16jj + y)
  - epilogue: hi+lo combine, stable softplus (Abs/Exp/Ln on ACT),
    noise scale+add, 3-way argmax on DVE, int32 cast, one [128, 32] DMA.
Token mapping: tok = u*4096 + jj*512 + c*128 + p; host inverse-permutes
the [8, 128, 32] device output back to token order.
"""

from collections import deque

import numpy as np

N = 262144
D = 256
E = 3
NCORES = 8
NPC = N // NCORES          # 32768 tokens per core
NT = 8                     # tiles per core
NB = 8                     # 512-token blocks (bands) per tile
TB = 512                   # tokens per block
NC4 = 4                    # 128-token chunks per block
BW = 16                    # band width in PSUM rows
NOISE_EPS = 0.2

_CACHE = {}


def _patched_act_tables():
    """Force Exp and Ln onto the combined natural_log_exp_and_others table.

    The act-table load pass assigns each activation the first table
    containing its function, so Exp -> exp_and_others and Ln -> natural_log,
    ping-ponging a ~1.3us table load per use. Stripping exp/ln from every
    table except the combined one (positions preserved, so set ids stay
    valid) leaves one load for the whole kernel.
    """
    from contextlib import contextmanager

    import concourse.bacc as bacc
    import concourse.mybir as mybir

    @contextmanager
    def ctx():
        orig = bacc.get_activation_tables

        def patched(arch):
            tables = dict(orig(arch))
            Act = mybir.ActivationFunctionType
            out = {}
            for name, funcs in tables.items():
                if name != "natural_log_exp_and_others":
                    funcs = funcs - {Act.Exp, Act.Ln}
                out[name] = funcs
            return out

        bacc.get_activation_tables = patched
        try:
            yield
        finally:
            bacc.get_activation_tables = orig

    return ctx()


def _build(variant="full", repeat=1, in_bufs=3, l_bufs=2, f_bufs=2,
           ep_bufs=2, mm_lag=2, post_lag=3, dma_engines=("sync", "scalar"),
           noise_eng="sync", out_eng="scalar", tpd=2, stag=False):
    from contextlib import ExitStack

    import concourse.bacc as bacc
    import concourse.mybir as mybir
    import concourse.tile as tile

    dt = mybir.dt
    Alu = mybir.AluOpType
    Act = mybir.ActivationFunctionType
    if isinstance(dma_engines, str):
        dma_engines = tuple(dma_engines.split("+"))
    do_mm = variant in ("full", "no_epilogue", "mm_only", "pe_only", "ov")
    do_post = variant in ("full", "no_epilogue")
    do_ep = variant == "full"

    nc = bacc.Bacc(
        "TRN2",
        target_bir_lowering=False,
        debug=False,
        enable_asserts=False,
        num_devices=NCORES,
    )
    # input, pre-transposed+fp16 on host: [D, NPC], row d, col token
    inT = nc.dram_tensor("inT", [D, NPC], dt.float16, kind="ExternalInput").ap()
    # wpack[p, (k*2+h)*6 + s*3 + e]: w_{gate|noise}_{hi|lo}[k*128+p, e]
    wpk = nc.dram_tensor("wpack", [128, 24], dt.float16, kind="ExternalInput").ap()
    # noiseF[p, u*96 + (c*NB+j)*3 + e] = noise[tok(u,j,c,p), e]
    noi = nc.dram_tensor("noiseF", [128, NT * NC4 * NB * E], dt.float16,
                         kind="ExternalInput").ap()
    # G: transpose-and-combine matrix. F = Ls.T @ G per 128-chunk:
    # G[16jj+y, 16jj+y] = G[16jj+6+y, 16jj+y] = 1 (y<6) folds the hi+lo
    # add into the PE transpose-back.
    idd = nc.dram_tensor("gmat", [128, 128], dt.float32, kind="ExternalInput").ap()
    # out[u, p, c*NB + j] = argmax for tok(u,j,c,p)
    out = nc.dram_tensor("out", [NT, 128, NC4 * NB], dt.int32,
                         kind="ExternalOutput").ap()
    # DCE-anchor scratch for ablation variants (not created in "full")
    scr = None
    if variant != "full":
        scr = nc.dram_tensor("scratch", [NT, 128, NC4 * NB], dt.float16,
                             kind="ExternalOutput").ap()

    # DMA view (tpd tiles per DMA, tpd*2 MiB per DMA):
    # inp_r[v, p, k, t2] = inT[k*128 + p, v*tpd*4096 + t2]
    inp_r = inT.rearrange("(k p) (v t) -> v p k t", k=2, v=NT // tpd)

    with tile.TileContext(nc) as tc, ExitStack() as ctx:
        const_pool = ctx.enter_context(tc.tile_pool(name="const", bufs=1))
        in_pool = ctx.enter_context(tc.tile_pool(name="inp", bufs=in_bufs))
        lsum_pool = ctx.enter_context(tc.tile_pool(name="lsum", bufs=l_bufs, space="PSUM"))
        ls_pool = ctx.enter_context(tc.tile_pool(name="ls", bufs=2))
        fsum_pool = ctx.enter_context(tc.tile_pool(name="fsum", bufs=f_bufs, space="PSUM"))
        ep_pool = ctx.enter_context(tc.tile_pool(name="ep", bufs=ep_bufs))
        noise_pool = ctx.enter_context(tc.tile_pool(name="noise", bufs=2))
        outp_pool = ctx.enter_context(tc.tile_pool(name="outp", bufs=2))

        f32r = dt.float32r
        ident = const_pool.tile([128, 128], f32r)
        nc.sync.dma_start(ident[:], idd.bitcast(f32r))
        wp = const_pool.tile([128, 24], dt.float16)
        nc.sync.dma_start(wp[:], wpk)

        # stationaries: stat[:, (jj*2+k)*128 : +128] has the 12 w cols at
        # column offset 16*jj, zeros elsewhere
        stat = const_pool.tile([128, 16 * 128], dt.float16)
        nc.vector.memset(stat[:], 0.0)
        for jj in range(NB):
            for k in range(2):
                base = (jj * 2 + k) * 128 + BW * jj
                for h in range(2):
                    for s in range(2):
                        nc.vector.tensor_copy(
                            stat[:, base + h * 6 + s * 3 : base + h * 6 + s * 3 + 3],
                            wp[:, (k * 2 + h) * 6 + s * 3 : (k * 2 + h) * 6 + s * 3 + 3],
                        )

        in_const = None
        if variant in ("pe_only", "ov"):
            in_const = const_pool.tile([128, 2 * tpd * 4096], dt.float16)
            nc.sync.dma_start(
                in_const[:].rearrange("p (k t) -> p k t", k=2), inp_r[0]
            )

        def emit_pair_dma(v):
            if in_const is not None:
                return in_const
            in_t = in_pool.tile([128, 2 * tpd * 4096], dt.float16)
            eng = getattr(nc, dma_engines[v % len(dma_engines)])
            eng.dma_start(in_t[:].rearrange("p (k t) -> p k t", k=2), inp_r[v])
            return in_t

        def emit_mm(ent):
            u, in_t, ul, nz = ent
            L = lsum_pool.tile([128, 512], dt.float32)
            for jj in range(NB):
                for k in range(2):
                    off = (k * tpd + ul) * 4096 + jj * TB
                    nc.tensor.matmul(
                        L[:],
                        lhsT=stat[:, (jj * 2 + k) * 128 : (jj * 2 + k) * 128 + 128],
                        rhs=in_t[:, off : off + TB],
                        start=(jj == 0 and k == 0),
                        stop=(jj == NB - 1 and k == 1),
                    )
            return u, L, nz

        def emit_post(ent):
            u, L, nz = ent
            if not do_post:
                return
            Ls = ls_pool.tile([128, 512], f32r)
            nc.scalar.copy(Ls[:], L[:].bitcast(f32r))
            F = fsum_pool.tile([128, 512], dt.float32)
            for c in range(NC4):
                nc.tensor.matmul(
                    F[:, c * 128 : c * 128 + 128],
                    lhsT=Ls[:, c * 128 : c * 128 + 128],
                    rhs=ident[:],
                    start=True,
                    stop=True,
                )
            if not do_ep:
                emit_probe(u, F[:, 0 : NC4 * NB])
                return
            Ff = F[:]
            # free layout: 128c + 16jj + y; y 0:3 = clean, 3:6 = raw
            # (hi+lo already combined by the G-matmul transpose)
            p16 = Ff.rearrange("p (c j y) -> p c j y", c=NC4, y=BW)
            clean3 = p16[:, :, :, 0:3]
            raw3 = p16[:, :, :, 3:6]
            # 2. stable softplus on raw: relu(x) + ln(1 + exp(-|x|))
            ab = ep_pool.tile([128, NC4 * NB * E], dt.float32)
            ab3 = ab[:].rearrange("p (c j e) -> p c j e", c=NC4, e=E)
            nc.scalar.activation(ab3, raw3, Act.Abs)
            ex = ep_pool.tile([128, NC4 * NB * E], dt.float32)
            nc.scalar.activation(ex[:], ab[:], Act.Exp, scale=-1.0)
            ln1p = ep_pool.tile([128, NC4 * NB * E], dt.float32)
            nc.scalar.activation(ln1p[:], ex[:], Act.Ln, bias=1.0)
            sp = ep_pool.tile([128, NC4 * NB * E], dt.float32)
            sp3 = sp[:].rearrange("p (c j e) -> p c j e", c=NC4, e=E)
            nc.vector.scalar_tensor_tensor(
                sp3, raw3, 0.0,
                ln1p[:].rearrange("p (c j e) -> p c j e", c=NC4, e=E),
                Alu.max, Alu.add,
            )
            # 3. t = (sp + eps) * noise ; logits = clean + t
            tt = ep_pool.tile([128, NC4 * NB * E], dt.float32)
            nc.vector.scalar_tensor_tensor(
                tt[:], sp[:], NOISE_EPS, nz, Alu.add, Alu.mult
            )
            lg = ep_pool.tile([128, NC4 * NB * E], dt.float32)
            lg3 = lg[:].rearrange("p (c j e) -> p c j e", c=NC4, e=E)
            nc.vector.tensor_tensor(
                lg3, clean3,
                tt[:].rearrange("p (c j e) -> p c j e", c=NC4, e=E),
                Alu.add,
            )
            # 4. 3-way argmax, first-max tie-break
            l0, l1, l2 = lg3[:, :, :, 0], lg3[:, :, :, 1], lg3[:, :, :, 2]
            c1 = ep_pool.tile([128, NC4 * NB], dt.float32)
            c14 = c1[:].rearrange("p (c j) -> p c j", c=NC4)
            nc.vector.tensor_tensor(c14, l1, l0, Alu.is_gt)
            mx = ep_pool.tile([128, NC4 * NB], dt.float32)
            mx4 = mx[:].rearrange("p (c j) -> p c j", c=NC4)
            nc.vector.tensor_tensor(mx4, l1, l0, Alu.max)
            c2 = ep_pool.tile([128, NC4 * NB], dt.float32)
            c24 = c2[:].rearrange("p (c j) -> p c j", c=NC4)
            nc.vector.tensor_tensor(c24, l2, mx4, Alu.is_gt)
            idxf = ep_pool.tile([128, NC4 * NB], dt.float32)
            nc.vector.scalar_tensor_tensor(
                idxf[:], c2[:], 2.0, c1[:], Alu.mult, Alu.max
            )
            idxi = outp_pool.tile([128, NC4 * NB], dt.int32)
            nc.vector.tensor_copy(idxi[:], idxf[:])
            getattr(nc, out_eng).dma_start(out[u], idxi[:])

        def emit_probe(u, src_ap):
            # DCE-proof consumer for ablation variants: forces the data to
            # be produced by writing a slice of it to the real output
            idxi = outp_pool.tile([128, NC4 * NB], dt.int32)
            nc.vector.tensor_copy(idxi[:], src_ap)
            getattr(nc, out_eng).dma_start(out[u], idxi[:])

        def build_iteration():
            mm_q = deque()
            post_q = deque()
            nztile = noise_pool.tile([128, NT * NC4 * NB * E], dt.float16)
            if variant != "pe_only":
                getattr(nc, noise_eng).dma_start(nztile[:], noi)

            def drain_mm():
                u, in_t, ul, nz = mm_q.popleft()
                if not do_mm:
                    emit_probe(u, in_t[:, ul * 4096 : ul * 4096 + NC4 * NB].bitcast(dt.float16))
                    return
                if variant == "ov":
                    # free-running stream DMA anchored via direct DRAM copy-out
                    nc.gpsimd.dma_start(
                        scr[u], in_t[:, ul * 4096 : ul * 4096 + NC4 * NB]
                    )
                    in_t = in_const
                post_q.append(emit_mm((u, in_t, ul, nz)))

            def drain_post():
                u, L, nz = post_q.popleft()
                if variant in ("mm_only", "pe_only", "ov"):
                    emit_probe(u, L[:, 0 : NC4 * NB])
                    return
                emit_post((u, L, nz))

            for v in range(NT // tpd):
                in_t = emit_pair_dma(v)
                for ul in range(tpd):
                    u = tpd * v + ul
                    nz = nztile[:, u * NC4 * NB * E : (u + 1) * NC4 * NB * E]
                    mm_q.append((u, in_t, ul, nz))
                    if len(mm_q) > mm_lag:
                        drain_mm()
                    if len(post_q) > post_lag - mm_lag:
                        drain_post()
            while mm_q:
                drain_mm()
            while post_q:
                drain_post()

        if repeat > 1:
            with tc.For_i(0, repeat, 1, staggered_reset=stag):
                build_iteration()
        else:
            build_iteration()

    with _patched_act_tables():
        nc.compile()
    return nc


BEST = dict(
    in_bufs=3,
    l_bufs=2,
    f_bufs=2,
    ep_bufs=2,
    mm_lag=2,
    post_lag=3,
    # single HWDGE ring for the fat input stream: chunks complete one at a
    # time at full rate, so the matmuls chase them with no idle ramp pairs;
    # noise/output DMAs ride the independent SWDGE queue so their semaphore
    # waits never block the input ring FIFO.
    dma_engines="sync",
    noise_eng="gpsimd",
    out_eng="gpsimd",
    tpd=2,
)


def _get_nc():
    if "nc" not in _CACHE:
        _CACHE["nc"] = _build(**BEST)
    return _CACHE["nc"]


def _run(in_maps, trace=False):
    from concourse.bass_utils import run_bass_kernel_spmd

    nc = _get_nc()
    return run_bass_kernel_spmd(nc, in_maps, list(range(NCORES)), trace=trace)


def _make_in_maps(input, w_gate, w_noise, noise):
    input = np.asarray(input, dtype=np.float32)
    noise = np.asarray(noise, dtype=np.float32)
    w_gate = np.asarray(w_gate, dtype=np.float32)
    w_noise = np.asarray(w_noise, dtype=np.float32)
    gmat = np.zeros((128, 128), dtype=np.float32)
    for jj in range(NB):
        for y in range(6):
            gmat[BW * jj + y, BW * jj + y] = 1.0
            gmat[BW * jj + 6 + y, BW * jj + y] = 1.0

    # wpack [128, 24] fp16: col (k*2+h)*6 + s*3 + e
    wpack = np.zeros((128, 24), np.float16)
    for s, w in enumerate((w_gate, w_noise)):
        hi = w.astype(np.float16)
        lo = (w - hi.astype(np.float32)).astype(np.float16)
        for k in range(2):
            for h, wh in enumerate((hi, lo)):
                wpack[:, (k * 2 + h) * 6 + s * 3 : (k * 2 + h) * 6 + s * 3 + 3] = (
                    wh[k * 128 : (k + 1) * 128]
                )

    in16 = input.astype(np.float16)
    in_maps = []
    for cid in range(NCORES):
        sl = slice(cid * NPC, (cid + 1) * NPC)
        # [NPC, D] -> [D, NPC] contiguous
        inT = np.ascontiguousarray(in16[sl].T)
        # noiseF[p, u*96 + (c*NB+j)*3+e] = noise[u*4096 + j*512 + c*128 + p, e]
        nF = np.ascontiguousarray(
            noise[sl].astype(np.float16)
            .reshape(NT, NB, NC4, 128, E).transpose(3, 0, 2, 1, 4)
        ).reshape(128, NT * NC4 * NB * E)
        in_maps.append(
            {"inT": inT, "noiseF": nF, "wpack": wpack, "gmat": gmat}
        )
    return in_maps


def kernel(input, w_gate, w_noise, noise):
    res = _run(_make_in_maps(input, w_gate, w_noise, noise))
    outs = []
    for r in res.results:
        o = r["out"]  # [NT, 128, NC4*NB]
        # out[u, p, c*NB + j] -> token u*4096 + j*512 + c*128 + p
        outs.append(
            np.ascontiguousarray(
                o.reshape(NT, 128, NC4, NB).transpose(0, 3, 2, 1)
            ).reshape(NPC)
        )
    return np.concatenate(outs, axis=0).astype(np.int32)
